# revision 2
# baseline (speedup 1.0000x reference)
"""GAT conv layer (B=2, N=4096, C=256, H=4, D=64) on TRN2 — loop-ified v2.

Execution-environment reality (measured via microbenchmarks):
  - cost ~50-100us per EMITTED program instruction (dispatch/translation),
    roughly independent of engine and tile size;
  - For_i hardware-loop iterations are ~free (re-execution at native speed);
  - cores execute in parallel (marginal cost flat 1..8 cores).
So v2 minimizes PROGRAM SIZE (~150 instructions vs ~1850 in v1) by wrapping
every phase in For_i loops. All dynamic addressing is confined to DMA dram
indices (ds() offsets); matmul stationary operands live at fixed SBUF
addresses and are refreshed by DMA each iteration; PSUM accumulates with
start=False over an explicit memset.

Per core (batch b):
  A: Wh|1 + tgt projection, loop over 32 j-chunks -> waugd dram
  B: src_i broadcast rows via replicated-column stationary, loop over i
  C: 4 i-quarters x For_i(32 j-chunks): mask dma, z=(src+tgt)-m255,
     l=leaky, p=exp(l) fp16, acc[65,(h,1024)] += (Wh|1).T @ p
  D: For_i(32 i-blocks): PE-transpose [65,128]->[128,65], out_acc +=
     num/den per head (DVE divide with per-partition denominator scalar)
Output [32, 128, 64] = row-blocks of [N, D]; host just reshapes.
"""

import numpy as np

B, N, C, H, D = 2, 4096, 256, 4, 64
NEG = 0.2
JC = N // 128          # 32 j-chunks
IQ4 = 4                # i quarters
WI = N // IQ4          # 1024 i per quarter
WCOL = H * 65 + H      # 264: [0.25*W_h | w_tgt_h]*4 + dup tgt cols

_cached = {}


def _build(reps=1):
    import concourse.bacc as bacc
    import concourse.tile as tile
    from concourse import mybir
    from concourse.masks import make_identity
    from concourse.bass import ds

    f32 = mybir.dt.float32
    f16 = mybir.dt.float16
    u8 = mybir.dt.uint8
    Alu = mybir.AluOpType

    nc = bacc.Bacc(None, target_bir_lowering=False, name="gatv2")

    xTd = nc.dram_tensor("xTd", [2, 128, N], f32, kind="ExternalInput")
    waugin = nc.dram_tensor("waugin", [2, 128, WCOL], f32, kind="ExternalInput")
    wsbd = nc.dram_tensor("wsbd", [H, 2, 128, 128], f32, kind="ExternalInput")
    mprep = nc.dram_tensor("mprep", [JC, IQ4, 128, WI], u8, kind="ExternalInput")
    outd = nc.dram_tensor("out", [JC, 128, D], f32, kind="ExternalOutput")

    waugd = nc.dram_tensor("waugd", [JC, 128, WCOL], f16, kind="Internal")
    srcBd = nc.dram_tensor("srcBd", [H, 8, 128, 512], f32, kind="Internal")
    ndd = nc.dram_tensor("ndd", [H, 65, N], f32, kind="Internal")

    def pipeline(tc):
        with tc.tile_pool(name="ld", bufs=1) as ld:
            ident = ld.tile([65, 65], f32)
            make_identity(nc, ident)
            xT_sb = ld.tile([128, 2 * N], f32)
            waug_sb = ld.tile([128, 2 * WCOL], f32)
            wsb_sb = ld.tile([128, H * 2 * 128], f32)
            for cc in range(2):
                nc.sync.dma_start(xT_sb[:, cc * N:(cc + 1) * N], xTd[cc])
                nc.sync.dma_start(
                    waug_sb[:, cc * WCOL:(cc + 1) * WCOL], waugin[cc])
                for h in range(H):
                    nc.sync.dma_start(
                        wsb_sb[:, (h * 2 + cc) * 128:(h * 2 + cc + 1) * 128],
                        wsbd[h, cc])

            # ---------------- phase A: project [Wh|tgt] per j-chunk -------
            with tc.tile_pool(name="pA", bufs=1) as pA, \
                 tc.tile_pool(name="psA", bufs=1, space="PSUM") as psA:
                xstage = pA.tile([128, 2 * 128], f32)
                stage16 = pA.tile([128, WCOL], f16)
                psp = psA.tile([128, WCOL], f32)
                with tc.For_i(0, JC) as jc:
                    for cc in range(2):
                        nc.sync.dma_start(
                            xstage[:, cc * 128:(cc + 1) * 128],
                            xTd[cc, :, ds(jc * 128, 128)])
                    for cc in range(2):
                        nc.tensor.matmul(
                            psp,
                            xstage[:, cc * 128:(cc + 1) * 128],
                            waug_sb[:, cc * WCOL:(cc + 1) * WCOL],
                            start=(cc == 0), stop=(cc == 1))
                    nc.vector.tensor_copy(stage16, psp)
                    nc.vector.memset(
                        stage16[:, 0:H * 65].rearrange(
                            "p (h l) -> p h l", l=65)[:, :, 64:65], 1.0)
                    nc.sync.dma_start(waugd[ds(jc, 1)], stage16)

            # ---------------- phase B: srcB rows via bcast matmul ---------
            srcB = ld.tile([128, H * N], f16)
            with tc.tile_pool(name="psB", bufs=1, space="PSUM") as psB:
                pss = [psB.tile([128, 512], f32, name=f"pss{h}", tag=f"pss{h}")
                       for h in range(H)]
                with tc.For_i(0, 8) as iq:
                    for h in range(H):
                        for cc in range(2):
                            nc.tensor.matmul(
                                pss[h],
                                wsb_sb[:, (h * 2 + cc) * 128:
                                       (h * 2 + cc + 1) * 128],
                                xT_sb[:, ds(cc * N + iq * 512, 512)],
                                start=(cc == 0), stop=(cc == 1),
                                skip_group_check=True)
                        nc.vector.tensor_copy(
                            srcB[:, ds(h * N + iq * 512, 512)], pss[h])

            # ---------------- phase C: scores + attention matmul ----------
            with tc.tile_pool(name="pC", bufs=1) as pC, \
                 tc.tile_pool(name="psC", bufs=1, space="PSUM") as psC:
                m_t = pC.tile([128, WI], u8)
                wstage = pC.tile([128, WCOL], f16)
                z4 = pC.tile([128, H * WI], f16)
                l4 = pC.tile([128, H * WI], f16)
                p4 = pC.tile([128, H * WI], f16)
                acc = psC.tile([65, H * WI], f32)
                for iq4 in range(IQ4):
                    mprep_v = mprep[:, iq4]
                    nc.vector.memset(acc, 0.0)
                    with tc.For_i(0, JC) as jc:
                        nc.sync.dma_start(m_t, mprep_v[ds(jc, 1)])
                        nc.sync.dma_start(wstage, waugd[ds(jc, 1)])
                        for h in range(H):
                            nc.vector.scalar_tensor_tensor(
                                out=z4[:, h * WI:(h + 1) * WI],
                                in0=srcB[:, h * N + iq4 * WI:
                                         h * N + (iq4 + 1) * WI],
                                scalar=wstage[:, H * 65 + h:H * 65 + h + 1],
                                in1=m_t,
                                op0=Alu.add, op1=Alu.subtract)
                        nc.vector.scalar_tensor_tensor(
                            out=l4, in0=z4, scalar=NEG, in1=z4,
                            op0=Alu.mult, op1=Alu.max)
                        nc.scalar.activation(
                            out=p4, in_=l4,
                            func=mybir.ActivationFunctionType.Exp)
                        for h in range(H):
                            for q in range(WI // 512):
                                nc.tensor.matmul(
                                    acc[:, h * WI + q * 512:
                                        h * WI + (q + 1) * 512],
                                    wstage[:, h * 65:(h + 1) * 65],
                                    p4[:, h * WI + q * 512:
                                       h * WI + (q + 1) * 512],
                                    start=False, stop=False,
                                    skip_group_check=True)
                    nds = pC.tile([65, H * WI], f32, tag="nds")
                    nc.vector.tensor_copy(nds, acc)
                    for h in range(H):
                        nc.sync.dma_start(
                            ndd[h, :, iq4 * WI:(iq4 + 1) * WI],
                            nds[:, h * WI:(h + 1) * WI])

            # ---------------- phase D: transpose + divide + head mean -----
            with tc.tile_pool(name="pD", bufs=1) as pD, \
                 tc.tile_pool(name="psD", bufs=1, space="PSUM") as psD:
                nb = pD.tile([65, 128], f32)
                den = pD.tile([128, 1], f32)
                o_acc = pD.tile([128, D], f32)
                tr = psD.tile([128, 65], f32)
                with tc.For_i(0, JC) as blk:
                    nc.vector.memset(o_acc, 0.0)
                    for h in range(H):
                        nc.sync.dma_start(nb, ndd[h, :, ds(blk * 128, 128)])
                        nc.tensor.transpose(tr, nb, ident)
                        with nc.allow_low_precision(reason="softmax denom"):
                            nc.vector.reciprocal(den, tr[:, 64:65])
                        nc.vector.scalar_tensor_tensor(
                            out=o_acc, in0=tr[:, 0:D], scalar=den,
                            in1=o_acc, op0=Alu.mult, op1=Alu.add)
                    nc.sync.dma_start(outd[ds(blk, 1)], o_acc)

    with tile.TileContext(nc) as tc:
        for _rep in range(reps):
            pipeline(tc)

    nc.compile()
    return nc


def _prep_inputs(x, adj_matrix_masked, W, attention):
    """Host-side layout prep (slicing, transposes, weight packing)."""
    x = np.ascontiguousarray(x, dtype=np.float32)
    W = np.ascontiguousarray(W, dtype=np.float32)
    attention = np.ascontiguousarray(attention, dtype=np.float32)

    a_src = attention[:, :D, 0]          # [H, D]
    a_tgt = attention[:, D:, 0]          # [H, D]
    Wh_cols = W.reshape(C, H, D)
    w_src = np.einsum("chd,hd->ch", Wh_cols, a_src)   # [C, H]
    w_tgt = np.einsum("chd,hd->ch", Wh_cols, a_tgt)   # [C, H]

    waug = np.zeros((C, WCOL), np.float32)
    for h in range(H):
        waug[:, h * 65: h * 65 + 64] = 0.25 * Wh_cols[:, h, :]
        waug[:, h * 65 + 64] = w_tgt[:, h]
        waug[:, H * 65 + h] = w_tgt[:, h]
    waug = np.ascontiguousarray(waug.reshape(2, 128, WCOL))

    wsb = np.empty((H, 2, 128, 128), np.float32)
    for h in range(H):
        wsb[h] = np.repeat(w_src[:, h][:, None], 128, axis=1).reshape(2, 128, 128)

    in_maps = []
    for b in range(B):
        xTb = np.ascontiguousarray(x[b].T).reshape(2, 128, N)
        # mprep[jc, iq4, p, i] = 255*mask[b, 0, iq4*1024 + i, jc*128 + p]
        mb = adj_matrix_masked[b, 0]                   # [i, j] bool
        m = (mb.T.astype(np.uint8) * np.uint8(255))    # [j, i]
        m = m.reshape(JC, 128, IQ4, WI)                # jc, p, iq4, i
        m = np.ascontiguousarray(m.transpose(0, 2, 1, 3))
        in_maps.append(dict(xTd=xTb, waugin=waug, wsbd=wsb, mprep=m))
    return in_maps


def _run(x, adj_matrix_masked, W, attention, reps=1):
    from concourse.bass_utils import run_bass_kernel_spmd

    key = f"nc{reps}"
    if key not in _cached:
        _cached[key] = _build(reps)
    nc = _cached[key]

    in_maps = _prep_inputs(x, adj_matrix_masked, W, attention)
    res = run_bass_kernel_spmd(nc, in_maps, core_ids=[0, 1])
    out = np.empty((B, N, D), np.float32)
    for b in range(B):
        out[b] = res.results[b]["out"].reshape(N, D)
    return out, res


def kernel(x, adj_matrix_masked, W, attention):
    out, _ = _run(x, adj_matrix_masked, W, attention)
    return out
